# revision 1
# baseline (speedup 1.0000x reference)
"""Trainium2 Bass kernel for a GNN message-passing network.

Graph: N=100000 atoms (D=64), E=400000 edges, 3 message-passing steps with a
per-edge MLP (concat(src,dst) -> 128 -> 128 -> 64, relu everywhere), messages
segment-summed at src; then molecule readout (segment-sum by sorted mol_ids ->
MLP 64->256->256->256->1) over M=4000 molecules.

Distribution over 8 NeuronCores:
  - atoms partitioned into 8 uniform blocks of NB=12500; core c owns the edges
    whose src lies in block c (random graph => balanced within ~1%).
  - per step, each core computes A = states@Win[:64]+b_in and B = states@Win[64:]
    for its own atom block (bf16, 128-wide rows), AllGathers the B blocks, then
    per edge gathers A[src] (local) and B[dst] (from the gathered table) with
    dma_gather(transpose=True) giving feature-major [128, n] tiles that feed
    TensorE directly. Indices are int16, so all gather/scatter index spaces
    are kept < 32768 by construction (per-block indexing).
  - messages are scatter-added (f32) into the local atom block with
    dma_scatter_add. The CCE scatter-add loses updates on duplicate rows, so
    edges are grouped into per-(core,src) occurrence classes: within a class
    all rows are unique, classes are separated by barriers, and padding slots
    target a dedicated trash region (trailing-negative indices can't be used
    under SPMD because the valid count differs per core).
  - message-MLP weights are bf16 hi+lo pairs (two accumulated matmuls ~ f32
    weight precision); the tiny readout MLP runs in full f32.
  - readout: per-core molecule sums via a one-hot matmul into a 512-mol psum
    window (mol_ids are sorted, so each core's atom block spans < 512 mols),
    written to the global [4608,64] table with a unique-index scatter, then
    ReduceScatter and a per-core MLP on its 500 molecules. Host concatenates
    the 8 outputs.
"""

import os

import ml_dtypes
import numpy as np

N_ATOMS = 100000
N_EDGES = 400000
N_MOLS = 4000
D = 64
DM = 128
N_STEPS = 3
NC = 8
NB = N_ATOMS // NC          # 12500 atoms per core block
NBP = 12544                 # padded to 98*128 (>= NB+1; row NB is the trash row)
NCHUNK = NBP // 128         # 98
T = 512                     # edge tile (matmul moving width)
MOLP = 4608                 # padded mol table (fits any 512-mol window)
MOL_RS = N_MOLS // NC       # 500 molecules per core after ReduceScatter

_LAST = {}


def _wrap16(x, dtype=np.int16):
    """[L] -> [128, L//16] in the wrapped+replicated gather-index layout:
    edge i lives at partition i%16, column i//16; replicated to 8 groups."""
    L = x.shape[0]
    assert L % 16 == 0
    w = np.ascontiguousarray(x.reshape(L // 16, 16).T).astype(dtype)
    return np.ascontiguousarray(np.tile(w, (8, 1)))


def _host_prep(inputs):
    """Edge partitioning, ordering and index tensors.

    dma_scatter_add loses updates when two descriptors of one instruction (or
    of concurrently-running instructions) target the same row.  So edges are
    ordered by (occurrence-class, dst-bank, src), where the occurrence class
    r is the per-(core,src) occurrence rank of the edge.  All scatters of one
    class then have unique target rows; classes are separated by barriers.
    Padding scatter slots point at a per-instruction-unique trash region.
    """
    src = np.asarray(inputs["edge_src"]).astype(np.int64)
    dst = np.asarray(inputs["edge_dst"]).astype(np.int64)
    mol = np.asarray(inputs["mol_ids"]).astype(np.int64)

    core = src // NB
    bank = dst // NB

    # occurrence rank of each edge within (core, src); rank in hash-shuffled
    # order so each class spreads evenly over dst-banks
    shuf = (src * 2654435761 + dst * 40503) % 999983
    o1 = np.lexsort((shuf, src))
    occ = np.zeros(N_EDGES, dtype=np.int64)
    ssrc = src[o1]
    run_start = np.concatenate([[0], np.nonzero(np.diff(ssrc))[0] + 1])
    pos = np.arange(N_EDGES)
    starts = np.repeat(run_start, np.diff(np.concatenate([run_start,
                                                          [N_EDGES]])))
    occ[o1] = pos - starts
    R = int(occ.max()) + 1
    assert R <= 24

    # per (core, r, bank) counts -> uniform padded segment sizes P[r][j]
    key = (core * R + occ) * NC + bank
    cnt = np.bincount(key, minlength=NC * R * NC).reshape(NC, R, NC)
    P = np.zeros((R, NC), dtype=np.int64)
    for r in range(R):
        for j in range(NC):
            P[r, j] = -(-int(cnt[:, r, j].max()) // 128) * 128
        # round each class's total to a multiple of 128 is guaranteed;
        # bump the last bank so non-empty classes stay non-degenerate
        if P[r].sum() == 0:
            P[r, 0] = 128
    offs = np.zeros((R, NC), dtype=np.int64)
    acc_off = 0
    for r in range(R):
        for j in range(NC):
            offs[r, j] = acc_off
            acc_off += P[r, j]
    L = int(acc_off)

    order = np.lexsort((src, bank, occ + core * R))  # (core,(r,bank,src))
    gs = np.concatenate([[0], np.cumsum(
        np.bincount((core * R + occ) * NC + bank, minlength=NC * R * NC))])

    per_core = []
    for c in range(NC):
        a_idx = np.zeros(L, dtype=np.int64)          # pad: row 0 (valid, unused)
        b_idx = np.zeros(L, dtype=np.int64)
        s_idx = np.zeros(L, dtype=np.int64)
        for r in range(R):
            for j in range(NC):
                g = (c * R + r) * NC + j
                e = order[gs[g]:gs[g + 1]]
                n = e.shape[0]
                o = int(offs[r, j])
                assert n <= P[r, j]
                a_idx[o:o + n] = src[e] - c * NB
                b_idx[o:o + n] = dst[e] - j * NB
                # unique trash rows for this instruction's padding
                s_idx[o:o + P[r, j]] = NBP + np.arange(P[r, j])
                s_idx[o:o + n] = src[e] - c * NB
        assert a_idx.max() < NB and b_idx.max() < NB
        per_core.append({
            "aidx": _wrap16(a_idx),
            "bidx": _wrap16(b_idx),
            "sidx": _wrap16(s_idx),
        })

    # molecule one-hot aggregation data: local mol offset per atom (f32) and
    # the unique global scatter indices of the core's 512-mol window
    for c in range(NC):
        ms = mol[c * NB:(c + 1) * NB]
        mbase = int(ms[0])
        assert int(ms[-1]) - mbase < 512
        mf = np.full(NBP, 1000.0, dtype=np.float32)   # pads match nothing
        mf[:NB] = (ms - mbase).astype(np.float32)
        per_core[c]["molf"] = np.tile(
            mf.reshape(NCHUNK, 128).T.copy(), (1, 1))  # [128, 98]
        mrows = (mbase + np.arange(512)).astype(np.int16)
        per_core[c]["molrow"] = _wrap16(mrows)

    # initial states, feature-major + ones row, bf16: [65, NBP]
    st = np.asarray(inputs["atom_states"]).astype(np.float32)
    for c in range(NC):
        s0 = np.zeros((65, NBP), dtype=np.float32)
        s0[:D, :NB] = st[c * NB:(c + 1) * NB].T
        s0[D, :] = 1.0
        per_core[c]["st0"] = s0.astype(ml_dtypes.bfloat16)

    assert int(P.max()) <= 2048
    layout = {"R": R, "P": P.tolist(), "offs": offs.tolist(), "L": L}
    return per_core, layout


def _split_bf16(x):
    """x (f32) -> (hi, lo) bf16 with hi + lo ~= x (split-weight matmul trick)."""
    bf = ml_dtypes.bfloat16
    hi = x.astype(bf)
    lo = (x - hi.astype(np.float32)).astype(bf)
    return hi, lo


def _weights_prep(inputs):
    """Replicated weight tensors. Message-MLP weights are split into bf16
    hi+lo pairs (two accumulated matmuls ~ f32 weight precision); the tiny
    readout MLP runs in full f32."""
    w = {}
    wcat = np.zeros((N_STEPS, 65, 256), dtype=np.float32)
    for s in range(N_STEPS):
        wi = np.asarray(inputs["ms_W_in"][s], dtype=np.float32)   # [128,128]
        wcat[s, :D, :DM] = wi[:D]
        wcat[s, :D, DM:] = wi[D:]
        wcat[s, D, :DM] = np.asarray(inputs["ms_b_in"][s], dtype=np.float32)
    # SBUF layouts are partition-major: [partitions, step/index, free]
    tp = lambda x: np.ascontiguousarray(np.transpose(x, (1, 0, 2)))
    w["wcat"], w["wcat_lo"] = map(tp, _split_bf16(wcat))
    wh_hi, wh_lo = _split_bf16(np.asarray(inputs["ms_W_h"][:, 0], dtype=np.float32))
    w["wh"], w["wh_lo"] = tp(wh_hi), tp(wh_lo)
    w["bh"] = tp(np.asarray(inputs["ms_b_h"][:, 0], dtype=np.float32)[:, :, None])
    wo_hi, wo_lo = _split_bf16(np.asarray(inputs["ms_W_out"], dtype=np.float32))
    w["wout"], w["wout_lo"] = tp(wo_hi), tp(wo_lo)
    w["bout"] = tp(np.asarray(inputs["ms_b_out"], dtype=np.float32)[:, :, None])

    wro1 = np.zeros((65, 256), dtype=np.float32)
    wro1[:D] = np.asarray(inputs["ro_W_in"], dtype=np.float32)
    wro1[D] = np.asarray(inputs["ro_b_in"], dtype=np.float32)
    w["wro1"] = wro1
    wroh = np.asarray(inputs["ro_W_h"], dtype=np.float32)         # [2,256,256]
    w["wroh"] = np.ascontiguousarray(                             # [128,(l,kc,mc),128]
        wroh.reshape(2, 2, 128, 2, 128).transpose(2, 0, 1, 3, 4).reshape(128, 8, 128))
    w["broh"] = np.ascontiguousarray(                             # [128,(l,mc),1]
        np.asarray(inputs["ro_b_h"], dtype=np.float32).reshape(2, 2, 128)
        .transpose(2, 0, 1).reshape(128, 4, 1))
    w["wro2"] = np.ascontiguousarray(                             # [128,kc,1]
        np.asarray(inputs["ro_W_out"], dtype=np.float32).reshape(2, 128, 1)
        .transpose(1, 0, 2))
    w["bro2"] = np.asarray(
        inputs["ro_b_out"], dtype=np.float32).reshape(1, 1).copy()
    w["ident"] = np.eye(128, dtype=np.float32)
    w["iota512"] = np.tile(np.arange(512, dtype=np.float32), (128, 1))
    w["identb"] = np.eye(64, dtype=np.float32).astype(ml_dtypes.bfloat16)
    return w


def _build(layout):
    import concourse.bacc as bacc
    import concourse.bass as bass
    import concourse.mybir as mybir
    import concourse.tile as tile
    from concourse._compat import get_trn_type
    from concourse.library_config import mlp as mlp_lib

    F32 = mybir.dt.float32
    BF16 = mybir.dt.bfloat16
    I16 = mybir.dt.int16
    RELU = mybir.ActivationFunctionType.Relu
    ADD = mybir.AluOpType.add
    MAX = mybir.AluOpType.max

    R = layout["R"]
    P = layout["P"]
    offs = layout["offs"]
    L = layout["L"]
    L16 = L // 16
    ACCROWS = NBP + 2048

    dbg_skip = set(os.environ.get("GNN_SKIP", "").split(","))

    nc = bacc.Bacc(get_trn_type() or "TRN2", target_bir_lowering=False)

    # ---- I/O -------------------------------------------------------------
    st0_d = nc.dram_tensor("st0", [65, NBP], BF16, kind="ExternalInput")
    aidx_d = nc.dram_tensor("aidx", [128, L16], I16, kind="ExternalInput")
    bidx_d = nc.dram_tensor("bidx", [128, L16], I16, kind="ExternalInput")
    sidx_d = nc.dram_tensor("sidx", [128, L16], I16, kind="ExternalInput")
    molf_d = nc.dram_tensor("molf", [128, NCHUNK], F32, kind="ExternalInput")
    molrow_d = nc.dram_tensor("molrow", [128, 32], I16, kind="ExternalInput")
    iota_d = nc.dram_tensor("iota512", [128, 512], F32, kind="ExternalInput")
    wcat_d = nc.dram_tensor("wcat", [65, N_STEPS, 256], BF16, kind="ExternalInput")
    wcatl_d = nc.dram_tensor("wcat_lo", [65, N_STEPS, 256], BF16, kind="ExternalInput")
    wh_d = nc.dram_tensor("wh", [128, N_STEPS, 128], BF16, kind="ExternalInput")
    whl_d = nc.dram_tensor("wh_lo", [128, N_STEPS, 128], BF16, kind="ExternalInput")
    bh_d = nc.dram_tensor("bh", [128, N_STEPS, 1], F32, kind="ExternalInput")
    wout_d = nc.dram_tensor("wout", [128, N_STEPS, 64], BF16, kind="ExternalInput")
    woutl_d = nc.dram_tensor("wout_lo", [128, N_STEPS, 64], BF16, kind="ExternalInput")
    bout_d = nc.dram_tensor("bout", [64, N_STEPS, 1], F32, kind="ExternalInput")
    wro1_d = nc.dram_tensor("wro1", [65, 256], F32, kind="ExternalInput")
    wroh_d = nc.dram_tensor("wroh", [128, 8, 128], F32, kind="ExternalInput")
    broh_d = nc.dram_tensor("broh", [128, 4, 1], F32, kind="ExternalInput")
    wro2_d = nc.dram_tensor("wro2", [128, 2, 1], F32, kind="ExternalInput")
    bro2_d = nc.dram_tensor("bro2", [1, 1], F32, kind="ExternalInput")
    ident_d = nc.dram_tensor("ident", [128, 128], F32, kind="ExternalInput")
    identb_d = nc.dram_tensor("identb", [64, 64], BF16, kind="ExternalInput")
    out_d = nc.dram_tensor("mol_out", [512], F32, kind="ExternalOutput")
    n_steps_run = int(os.environ.get("GNN_STEPS", "3"))
    debug = os.environ.get("GNN_DEBUG", "0") == "1"
    if debug:
        dbga_d = nc.dram_tensor("dbg_a", [NBP, 128], BF16, kind="ExternalOutput")
        dbgb_d = nc.dram_tensor("dbg_b", [NBP, 128], BF16, kind="ExternalOutput")
        dbgacc_d = nc.dram_tensor("dbg_acc", [NBP, 64], F32, kind="ExternalOutput")

    # ---- internal DRAM ---------------------------------------------------
    a_tab = nc.dram_tensor("a_tab", [NBP, 128], BF16)
    b_loc = nc.dram_tensor("b_loc", [NBP, 128], BF16)
    b_full = nc.dram_tensor("b_full", [NC * NBP, 128], BF16, addr_space="Shared")
    acc = nc.dram_tensor("acc", [NBP + 2048, 64], F32)
    mol_acc = nc.dram_tensor("mol_acc", [MOLP, 64], F32)
    mol_rs = nc.dram_tensor("mol_rs", [MOL_RS, 64], F32)

    with tile.TileContext(nc) as tc:
        cpool = tc.alloc_tile_pool(name="consts", bufs=1)

        nc.gpsimd.load_library(mlp_lib)

        # persistent SBUF state / constants
        st_fm = cpool.tile([65, NBP], BF16, tag="st_fm")
        aidx = cpool.tile([128, L16], I16, tag="aidx")
        bidx = cpool.tile([128, L16], I16, tag="bidx")
        sidx = cpool.tile([128, L16], I16, tag="sidx")
        molf = cpool.tile([128, NCHUNK], F32, tag="molf")
        molrow = cpool.tile([128, 32], I16, tag="molrow")
        iota512 = cpool.tile([128, 512], F32, tag="iota512")
        wcat = cpool.tile([65, N_STEPS, 256], BF16, tag="wcat")
        wcatl = cpool.tile([65, N_STEPS, 256], BF16, tag="wcatl")
        wh = cpool.tile([128, N_STEPS, 128], BF16, tag="wh")
        whl = cpool.tile([128, N_STEPS, 128], BF16, tag="whl")
        bh = cpool.tile([128, N_STEPS, 1], F32, tag="bh")
        wout = cpool.tile([128, N_STEPS, 64], BF16, tag="wout")
        woutl = cpool.tile([128, N_STEPS, 64], BF16, tag="woutl")
        bout = cpool.tile([64, N_STEPS, 1], F32, tag="bout")
        wro1 = cpool.tile([65, 256], F32, tag="wro1")
        wroh = cpool.tile([128, 8, 128], F32, tag="wroh")
        broh = cpool.tile([128, 4, 1], F32, tag="broh")
        wro2 = cpool.tile([128, 2, 1], F32, tag="wro2")
        bro2 = cpool.tile([1, 1], F32, tag="bro2")
        ident = cpool.tile([128, 128], F32, tag="ident")
        identb = cpool.tile([64, 64], BF16, tag="identb")
        zeros = cpool.tile([128, 512], F32, tag="zeros")

        for sb, dr in [(st_fm, st0_d), (aidx, aidx_d), (bidx, bidx_d),
                       (sidx, sidx_d), (molf, molf_d), (molrow, molrow_d),
                       (iota512, iota_d), (wcat, wcat_d),
                       (wcatl, wcatl_d), (wh, wh_d), (whl, whl_d),
                       (bh, bh_d), (wout, wout_d), (woutl, woutl_d),
                       (bout, bout_d),
                       (wro1, wro1_d), (wroh, wroh_d), (broh, broh_d),
                       (wro2, wro2_d), (bro2, bro2_d), (ident, ident_d),
                       (identb, identb_d)]:
            nc.sync.dma_start(sb[:], dr[:])
        nc.vector.memset(zeros[:], 0.0)

        def zero_dram(dram, nrows):
            full = nrows // 1024
            for r in range(full):
                nc.sync.dma_start(dram[r * 1024:(r + 1) * 1024, :], zeros[:])
            rr = nrows - full * 1024
            if rr:
                assert (rr * 64) % 128 == 0
                nc.sync.dma_start(dram[full * 1024:nrows, :],
                                  zeros[:, 0:(rr * 64) // 128])

        zero_dram(mol_acc, MOLP)

        for step in range(n_steps_run):
            with tc.tile_pool(name=f"w{step}", bufs=2) as wpool:
                if step > 0:
                    # transpose acc [NBP,64] f32 -> st_fm[0:64] bf16
                    with tc.tile_pool(name=f"ptr{step}", bufs=2,
                                      space="PSUM") as tpool:
                        for g in range(NCHUNK // 7):  # 14 groups of 7 chunks
                            tin = wpool.tile([128, 7, 64], F32, tag="tin")
                            nc.sync.dma_start(
                                tin[:],
                                acc[g * 896:(g + 1) * 896, :].rearrange(
                                    "(i p) c -> p i c", p=128))
                            for i in range(7):
                                c = g * 7 + i
                                ptr = tpool.tile([64, 128], F32, tag="ptr")
                                nc.tensor.matmul(ptr[:], tin[:, i, :],
                                                 ident[:], is_transpose=True,
                                                 start=True, stop=True)
                                nc.vector.tensor_copy(
                                    st_fm[0:64, c * 128:(c + 1) * 128], ptr[:])
                    tc.strict_bb_all_engine_barrier()

                # A/B tables for own block
                with tc.tile_pool(name=f"pab{step}", bufs=2,
                                  space="PSUM") as apool:
                    for g in range(NCHUNK // 7):
                        asb = wpool.tile([128, 7, 128], BF16, tag="asb")
                        bsb = wpool.tile([128, 7, 128], BF16, tag="bsb")
                        for i in range(7):
                            c = g * 7 + i
                            pab = apool.tile([128, 256], F32, tag="pab")
                            nc.tensor.matmul(
                                pab[:], st_fm[:, c * 128:(c + 1) * 128],
                                wcat[:, step, :], start=True, stop=False)
                            nc.tensor.matmul(
                                pab[:], st_fm[:, c * 128:(c + 1) * 128],
                                wcatl[:, step, :], start=False, stop=True)
                            nc.vector.tensor_copy(asb[:, i, :], pab[:, 0:128])
                            nc.scalar.activation(
                                bsb[:, i, :], pab[:, 128:256],
                                mybir.ActivationFunctionType.Copy)
                        nc.sync.dma_start(
                            a_tab[g * 896:(g + 1) * 896, :].rearrange(
                                "(i p) c -> p i c", p=128), asb[:])
                        nc.sync.dma_start(
                            b_loc[g * 896:(g + 1) * 896, :].rearrange(
                                "(i p) c -> p i c", p=128), bsb[:])
                    zero_dram(acc, NBP)
                tc.strict_bb_all_engine_barrier()

                if "cc" not in dbg_skip:
                    nc.gpsimd.collective_compute(
                        "AllGather", mybir.AluOpType.bypass,
                        replica_groups=[list(range(NC))],
                        ins=[b_loc[:]], outs=[b_full[:]])
                    tc.strict_bb_all_engine_barrier()

                # edge phase: occurrence classes (race-free scatters),
                # per class one gather/MLP/scatter chain per dst-bank
                ppool = tc.alloc_tile_pool(name=f"p{step}", bufs=2,
                                           space="PSUM")
                prev_sc, cur_sc = [], []
                for r in range(0 if "edge" not in dbg_skip else R, R):
                    for j in range(NC):
                        Prj = P[r][j]
                        if Prj == 0:
                            continue
                        o16 = offs[r][j] // 16
                        a_st = wpool.tile([128, 1, 2048], BF16, tag="a_st")
                        b_st = wpool.tile([128, 1, 2048], BF16, tag="b_st")
                        msg = wpool.tile([128, 16, 64], F32, tag="msg")
                        if "gather" not in dbg_skip:
                            nc.gpsimd.dma_gather(
                                a_st[:, :, 0:Prj], a_tab[:],
                                aidx[:, o16:o16 + Prj // 16],
                                Prj, Prj, 128, transpose=True,
                                single_packet=False)
                            nc.gpsimd.dma_gather(
                                b_st[:, :, 0:Prj],
                                b_full[j * NBP:(j + 1) * NBP, :],
                                bidx[:, o16:o16 + Prj // 16],
                                Prj, Prj, 128, transpose=True,
                                single_packet=False)
                        else:
                            nc.vector.memset(a_st[:], 0.0)
                            nc.vector.memset(b_st[:], 0.0)
                        for t0 in range(0, Prj, T):
                            w_t = min(T, Prj - t0)
                            sl = slice(t0, t0 + w_t)
                            h1 = wpool.tile([128, T], BF16, tag="h1")
                            nc.vector.tensor_tensor(
                                h1[:, 0:w_t], a_st[:, 0, sl], b_st[:, 0, sl],
                                ADD)
                            h1r = wpool.tile([128, T], BF16, tag="h1r")
                            nc.vector.tensor_scalar_max(
                                h1r[:, 0:w_t], h1[:, 0:w_t], 0.0)
                            p2 = ppool.tile([128, T], F32, tag="p2")
                            nc.tensor.matmul(p2[:, 0:w_t], wh[:, step, :],
                                             h1r[:, 0:w_t],
                                             start=True, stop=False)
                            nc.tensor.matmul(p2[:, 0:w_t], whl[:, step, :],
                                             h1r[:, 0:w_t],
                                             start=False, stop=True)
                            h2 = wpool.tile([128, T], BF16, tag="h2")
                            nc.scalar.activation(h2[:, 0:w_t], p2[:, 0:w_t],
                                                 RELU, bias=bh[:, step, :])
                            pm = ppool.tile([64, T], F32, tag="pm")
                            nc.tensor.matmul(pm[:, 0:w_t], wout[:, step, :],
                                             h2[:, 0:w_t],
                                             start=True, stop=False)
                            nc.tensor.matmul(pm[:, 0:w_t], woutl[:, step, :],
                                             h2[:, 0:w_t],
                                             start=False, stop=True)
                            mt = wpool.tile([64, T], BF16, tag="mt")
                            nc.vector.tensor_scalar(
                                mt[:, 0:w_t], pm[:, 0:w_t], bout[:, step, :],
                                0.0, op0=ADD, op1=MAX)
                            for k0 in range(0, w_t, 128):
                                pt = ppool.tile([128, 64], BF16, tag="pt")
                                nc.tensor.matmul(
                                    pt[:], mt[:, k0:k0 + 128],
                                    identb[:], is_transpose=True,
                                    start=True, stop=True)
                                nc.vector.tensor_copy(
                                    msg[:, (t0 + k0) // 128, :], pt[:])
                        if "scatter" not in dbg_skip:
                            si = nc.gpsimd.dma_scatter_add(
                                acc[:], msg[:, 0:Prj // 128, :],
                                sidx[:, o16:o16 + Prj // 16],
                                Prj, Prj, 64, single_packet=False)
                            # classes may hit the same rows: order scatter
                            # DMAs across classes, but let gathers/MLP of
                            # later classes overlap earlier scatters
                            for ps in prev_sc:
                                tile.add_dep_helper(
                                    si.ins, ps, sync=True,
                                    reason="scatter class serialization")
                            cur_sc.append(si.ins)
                    prev_sc, cur_sc = cur_sc, []
                ppool.release()
                tc.strict_bb_all_engine_barrier()

        if debug:
            for r in range(NBP // 1024 + 1):
                r0, r1 = r * 1024, min((r + 1) * 1024, NBP)
                nc.sync.dma_start(dbga_d[r0:r1, :], a_tab[r0:r1, :])
                nc.sync.dma_start(dbgb_d[r0:r1, :], b_loc[r0:r1, :])
                nc.sync.dma_start(dbgacc_d[r0:r1, :], acc[r0:r1, :])

        # ---- molecule partial sums (one-hot matmul into a 512-mol psum
        # window, then a unique-index scatter) + ReduceScatter --------------
        rpool = tc.alloc_tile_pool(name="ro", bufs=2)
        mpsum = tc.alloc_tile_pool(name="molp", bufs=2, space="PSUM")
        mps = mpsum.tile([64, 512], F32, tag="mps")
        nc.vector.memset(mps[:], 0.0)
        for g in range(NCHUNK // 7):
            stt = rpool.tile([128, 7, 64], F32, tag="stt")
            nc.sync.dma_start(stt[:],
                              acc[g * 896:(g + 1) * 896, :].rearrange(
                                  "(i p) c -> p i c", p=128))
            sttb = rpool.tile([128, 7, 64], BF16, tag="sttb")
            nc.vector.tensor_copy(sttb[:], stt[:])
            for i in range(7):
                ch = g * 7 + i
                moh = rpool.tile([128, 512], BF16, tag="moh")
                nc.vector.tensor_scalar(
                    moh[:], iota512[:], molf[:, ch:ch + 1], None,
                    op0=mybir.AluOpType.is_equal)
                nc.tensor.matmul(mps[:], sttb[:, i, :], moh[:],
                                 start=False, stop=False,
                                 skip_group_check=True)
        mfl = rpool.tile([64, 512], F32, tag="mfl")
        nc.vector.tensor_copy(mfl[:], mps[:])
        mws = rpool.tile([128, 4, 64], F32, tag="mws")
        for k in range(4):
            ptm = mpsum.tile([128, 64], F32, tag="ptm")
            nc.tensor.matmul(ptm[:], mfl[:, k * 128:(k + 1) * 128],
                             ident[0:64, 0:64], is_transpose=True,
                             start=True, stop=True)
            nc.vector.tensor_copy(mws[:, k, :], ptm[:])
        if "scatter" not in dbg_skip:
            nc.gpsimd.dma_scatter_add(
                mol_acc[:], mws[:], molrow[:, 0:32],
                512, 512, 64, single_packet=False)
        mpsum.release()
        rpsum = tc.alloc_tile_pool(name="rop", bufs=2, space="PSUM")
        tc.strict_bb_all_engine_barrier()
        if "cc" not in dbg_skip:
            nc.gpsimd.collective_compute(
                "ReduceScatter", mybir.AluOpType.add,
                replica_groups=[list(range(NC))],
                ins=[mol_acc[0:N_MOLS, :]], outs=[mol_rs[:]])
            tc.strict_bb_all_engine_barrier()

        # ---- readout MLP on the core's 500 molecules ---------------------
        mol_sb = rpool.tile([128, 4, 64], F32, tag="mol_sb")
        nc.sync.dma_start(mol_sb[:, 0:3, :],
                          mol_rs[0:384, :].rearrange(
                              "(i p) c -> p i c", p=128))
        nc.sync.dma_start(mol_sb[0:116, 3, :], mol_rs[384:500, :])
        molt = rpool.tile([65, 512], F32, tag="molt")
        nc.vector.memset(molt[:], 0.0)
        nc.vector.memset(molt[64:65, :], 1.0)
        for k in range(4):
            ptr = rpsum.tile([64, 128], F32, tag="rptr")
            nc.tensor.matmul(ptr[:], mol_sb[:, k, :], ident[:],
                             is_transpose=True, start=True, stop=True)
            nc.vector.tensor_copy(molt[0:64, k * 128:(k + 1) * 128], ptr[:])

        r_cur = rpool.tile([128, 2, 512], F32, tag="r_a")
        for mh in range(2):
            p = rpsum.tile([128, 512], F32, tag="rp")
            nc.tensor.matmul(p[:], wro1[:, mh * 128:(mh + 1) * 128],
                             molt[:], start=True, stop=True)
            nc.scalar.activation(r_cur[:, mh, :], p[:], RELU)
        for layer in range(2):
            r_nxt = rpool.tile([128, 2, 512], F32,
                               tag=("r_b" if layer == 0 else "r_a2"))
            for mh in range(2):
                p = rpsum.tile([128, 512], F32, tag="rp")
                nc.tensor.matmul(p[:], wroh[:, layer * 4 + mh, :], r_cur[:, 0, :],
                                 start=True, stop=False)
                nc.tensor.matmul(p[:], wroh[:, layer * 4 + 2 + mh, :], r_cur[:, 1, :],
                                 start=False, stop=True)
                nc.scalar.activation(r_nxt[:, mh, :], p[:], RELU,
                                     bias=broh[:, layer * 2 + mh, :])
            r_cur = r_nxt
        pout = rpsum.tile([1, 512], F32, tag="rpo")
        nc.tensor.matmul(pout[:], wro2[:, 0, :], r_cur[:, 0, :],
                         start=True, stop=False)
        nc.tensor.matmul(pout[:], wro2[:, 1, :], r_cur[:, 1, :],
                         start=False, stop=True)
        out_sb = rpool.tile([1, 512], F32, tag="out_sb")
        nc.vector.tensor_scalar_add(out_sb[:], pout[:], bro2[:])
        nc.sync.dma_start(out_d[:], out_sb[:])

        rpsum.release()
        rpool.release()
        cpool.release()

    nc.compile()
    return nc


def kernel(**inputs):
    from concourse.bass_utils import run_bass_kernel_spmd

    per_core, layout = _host_prep(inputs)
    w = _weights_prep(inputs)
    nc = _build(layout)

    in_maps = []
    for c in range(NC):
        m = dict(per_core[c])
        m.update(w)
        in_maps.append(m)

    trace = bool(int(os.environ.get("BASS_GNN_TRACE", "0")))
    res = run_bass_kernel_spmd(nc, in_maps, list(range(NC)), trace=trace)
    _LAST["exec_time_ns"] = res.exec_time_ns
    _LAST["results"] = res.results
    _LAST["res"] = res

    out = np.concatenate(
        [np.asarray(res.results[c]["mol_out"][:MOL_RS]) for c in range(NC)])
    return out.astype(np.float32).reshape(N_MOLS, 1)



# revision 30
# speedup vs baseline: 2.0182x; 2.0182x over previous
"""Trainium2 Bass kernel for a GNN message-passing network (v2).

Graph: N=100000 atoms (D=64), E=400000 edges, 3 message-passing steps with a
per-edge MLP (concat(src,dst) -> 128 -> 128 -> 64, relu everywhere), messages
segment-summed at src; then molecule readout (segment-sum by sorted mol_ids ->
MLP 64->256->256->256->1) over M=4000 molecules.

Distribution over 8 NeuronCores (v2 design):
  - atoms partitioned into 8 blocks of NB=12500; core c owns edges whose src
    is in block c. Edges are sorted by (src-chunk of 128 rows, dst 2-bank
    group), padded to a layout that is UNIFORM across cores (so one SPMD
    program works for all 8 cores).
  - per step each core computes A = st@Win[:64]+b and B = st@Win[64:] for its
    own rows (bf16, hi+lo split weights), AllGathers the B table.
  - The per-edge A[src] "gather" is a one-hot matmul: for each 128-row chunk
    k, h1 += A_k^T @ G_k where G_k[row, e] = [src_row(e) == row] is a static
    host-built one-hot streamed from DRAM.  B[dst] is fetched with
    dma_gather (the only dynamic indexing left), split into 4 gathers per
    wave over 2-bank windows (int16-index-safe) on 4 SWDGE queues so
    descriptor generation runs on all 8 Q7 cores, and added into the same
    PSUM tile with an identity matmul.
  - message aggregation at src is another one-hot matmul (E = G^T per
    128-edge subtile) accumulating in PSUM per chunk, writing the new states
    directly in feature-major layout -- no dma_scatter_add, no occurrence
    classes, no inter-step transpose.
  - final step aggregates row-major instead and feeds the molecule one-hot
    matmul (mol_ids sorted => each core block spans < 512 molecules), then
    a unique-index scatter into the global mol table, ReduceScatter, and a
    per-core readout MLP on its 500 molecules; host concatenates.
"""

import os

import ml_dtypes
import numpy as np

N_ATOMS = 100000
N_EDGES = 400000
N_MOLS = 4000
D = 64
DM = 128
N_STEPS = 3
NC = 8
NB = N_ATOMS // NC          # 12500 atoms per core block
NBP = 12544                 # padded to 98*128
NCHUNK = NBP // 128         # 98
NG = 4                      # dst 2-bank gather groups
WV = 8                      # chunks per wave
MOLP = 4608                 # padded mol table
MOL_RS = N_MOLS // NC       # 500 molecules per core after ReduceScatter
TS = 512                    # psum slice width

_LAST = {}


def _wrap16(x, dtype=np.int16):
    """[L] -> [128, L//16] wrapped+replicated gather-index layout."""
    L = x.shape[0]
    assert L % 16 == 0
    w = np.ascontiguousarray(x.reshape(L // 16, 16).T).astype(dtype)
    return np.ascontiguousarray(np.tile(w, (8, 1)))


def _host_prep(inputs):
    """Uniform edge layout + per-core index/one-hot tensors."""
    src = np.asarray(inputs["edge_src"]).astype(np.int64)
    dst = np.asarray(inputs["edge_dst"]).astype(np.int64)
    mol = np.asarray(inputs["mol_ids"]).astype(np.int64)

    core = src // NB
    srcl = src - core * NB
    chunk = srcl // 128
    row = srcl % 128
    bank = dst // NB
    grp = bank // 2
    idx16 = (bank % 2) * NBP + (dst - bank * NB)   # gather idx in 2-bank win
    assert idx16.max() < 2 * NBP <= 32768

    # per (core, chunk, grp) counts -> uniform LEN[k][g] = max over cores
    key = (core * NCHUNK + chunk) * NG + grp
    cnt = np.bincount(key, minlength=NC * NCHUNK * NG).reshape(NC, NCHUNK, NG)
    LEN = cnt.max(axis=0).astype(np.int64)         # [NCHUNK, NG]
    Tk = np.zeros(NCHUNK, dtype=np.int64)
    for k in range(NCHUNK):
        s = int(LEN[k].sum())
        Tk[k] = -(-s // 128) * 128
        LEN[k][NG - 1] += Tk[k] - s                # chunk pad -> last group
    POS = np.zeros(NCHUNK + 1, dtype=np.int64)     # chunk col base (global)
    np.cumsum(Tk, out=POS[1:])
    Lpad = int(POS[-1])

    # waves of WV chunks; per-(wave,grp) gather sizes (128-padded, uniform)
    waves = [list(range(w, min(w + WV, NCHUNK))) for w in range(0, NCHUNK, WV)]
    NW = []                                        # [wave][g] gather num_idxs
    BOFF = []                                      # [wave][g] -> {k: col off}
    for wch in waves:
        nw = []
        boffs = []
        for g in range(NG):
            off = {}
            acc = 0
            for k in wch:
                off[k] = acc
                acc += int(LEN[k][g])
            nw.append(-(-acc // 128) * 128)
            boffs.append(off)
        NW.append(nw)
        BOFF.append(boffs)
    # index-tensor column base per (wave, g), in edges (16-wrapped cols /16)
    IOFF = []
    acc = 0
    for w in range(len(waves)):
        offs = []
        for g in range(NG):
            offs.append(acc)
            acc += NW[w][g]
        IOFF.append(offs)
    LIDX = acc

    # order edges per core into the padded layout
    per_core = []
    for c in range(NC):
        sel = np.nonzero(core == c)[0]
        order = np.lexsort((srcl[sel], grp[sel], chunk[sel]))
        e = sel[order]
        ek, eg, erow, eidx = chunk[e], grp[e], row[e], idx16[e]

        bidx = np.zeros(LIDX, dtype=np.int64)
        G_all = np.zeros((128, Lpad), dtype=np.float32)
        E_all = np.zeros((128, Lpad), dtype=np.float32)
        # run starts per (k,g) in the sorted edge list
        ptr = 0
        for k in range(NCHUNK):
            for g in range(NG):
                n = int(cnt[c, k, g])
                run = slice(ptr, ptr + n)
                ptr += n
                w = k // WV
                # gather slots
                b0 = IOFF[w][g] + BOFF[w][g][k]
                bidx[b0:b0 + n] = eidx[run]
                # global edge positions
                p0 = POS[k] + sum(int(LEN[k][gg]) for gg in range(g))
                p = p0 + np.arange(n)
                r = erow[run]
                G_all[r, p] = 1.0
                E_all[p % 128, (p // 128) * 128 + r] = 1.0
        assert ptr == len(e)
        per_core.append({
            "bidx": _wrap16(bidx),
            "g_all": G_all.astype(ml_dtypes.bfloat16),
            "e_all": E_all.astype(ml_dtypes.bfloat16),
        })

    # molecule one-hot data (as baseline)
    for c in range(NC):
        ms = mol[c * NB:(c + 1) * NB]
        mbase = int(ms[0])
        assert int(ms[-1]) - mbase < 512
        mf = np.full(NBP, 1000.0, dtype=np.float32)
        mf[:NB] = (ms - mbase).astype(np.float32)
        per_core[c]["molf"] = np.ascontiguousarray(
            mf.reshape(NCHUNK, 128).T)                 # [128, 98]
        mrows = (mbase + np.arange(512)).astype(np.int16)
        per_core[c]["molrow"] = _wrap16(mrows)

    # initial states, feature-major + ones row, bf16: [65, NBP]
    st = np.asarray(inputs["atom_states"]).astype(np.float32)
    for c in range(NC):
        s0 = np.zeros((65, NBP), dtype=np.float32)
        s0[:D, :NB] = st[c * NB:(c + 1) * NB].T
        s0[D, :] = 1.0
        per_core[c]["st0"] = s0.astype(ml_dtypes.bfloat16)

    layout = {
        "LEN": LEN.tolist(), "Tk": Tk.tolist(), "POS": POS.tolist(),
        "waves": waves, "NW": NW, "BOFF": BOFF, "IOFF": IOFF,
        "LIDX": LIDX, "Lpad": Lpad,
    }
    return per_core, layout


def _split_bf16(x):
    bf = ml_dtypes.bfloat16
    hi = x.astype(bf)
    lo = (x - hi.astype(np.float32)).astype(bf)
    return hi, lo


def _weights_prep(inputs):
    """Replicated weight tensors (same scheme as baseline)."""
    w = {}
    wcat = np.zeros((N_STEPS, 65, 256), dtype=np.float32)
    for s in range(N_STEPS):
        wi = np.asarray(inputs["ms_W_in"][s], dtype=np.float32)   # [128,128]
        wcat[s, :D, :DM] = wi[:D]
        wcat[s, :D, DM:] = wi[D:]
        wcat[s, D, :DM] = np.asarray(inputs["ms_b_in"][s], dtype=np.float32)
    tp = lambda x: np.ascontiguousarray(np.transpose(x, (1, 0, 2)))
    w["wcat"], w["wcat_lo"] = map(tp, _split_bf16(wcat))
    wh_hi, wh_lo = _split_bf16(np.asarray(inputs["ms_W_h"][:, 0], dtype=np.float32))
    w["wh"], w["wh_lo"] = tp(wh_hi), tp(wh_lo)
    w["bh"] = tp(np.asarray(inputs["ms_b_h"][:, 0], dtype=np.float32)[:, :, None])
    wo_hi, wo_lo = _split_bf16(np.asarray(inputs["ms_W_out"], dtype=np.float32))
    w["wout"], w["wout_lo"] = tp(wo_hi), tp(wo_lo)
    w["bout"] = tp(np.asarray(inputs["ms_b_out"], dtype=np.float32)[:, :, None])

    wro1 = np.zeros((65, 256), dtype=np.float32)
    wro1[:D] = np.asarray(inputs["ro_W_in"], dtype=np.float32)
    wro1[D] = np.asarray(inputs["ro_b_in"], dtype=np.float32)
    w["wro1"] = wro1
    wroh = np.asarray(inputs["ro_W_h"], dtype=np.float32)         # [2,256,256]
    w["wroh"] = np.ascontiguousarray(
        wroh.reshape(2, 2, 128, 2, 128).transpose(2, 0, 1, 3, 4).reshape(128, 8, 128))
    w["broh"] = np.ascontiguousarray(
        np.asarray(inputs["ro_b_h"], dtype=np.float32).reshape(2, 2, 128)
        .transpose(2, 0, 1).reshape(128, 4, 1))
    w["wro2"] = np.ascontiguousarray(
        np.asarray(inputs["ro_W_out"], dtype=np.float32).reshape(2, 128, 1)
        .transpose(1, 0, 2))
    w["bro2"] = np.asarray(
        inputs["ro_b_out"], dtype=np.float32).reshape(1, 1).copy()
    w["ident"] = np.eye(128, dtype=np.float32)
    w["iota512"] = np.tile(np.arange(512, dtype=np.float32), (128, 1))
    w["identb"] = np.eye(64, dtype=np.float32).astype(ml_dtypes.bfloat16)
    w["identb128"] = np.eye(128, dtype=np.float32).astype(ml_dtypes.bfloat16)
    return w


def _build(layout):
    import concourse.bacc as bacc
    import concourse.bass as bass
    import concourse.mybir as mybir
    import concourse.tile as tile
    from concourse._compat import get_trn_type
    from concourse.library_config import mlp as mlp_lib

    F32 = mybir.dt.float32
    BF16 = mybir.dt.bfloat16
    I16 = mybir.dt.int16
    RELU = mybir.ActivationFunctionType.Relu
    COPY = mybir.ActivationFunctionType.Copy
    ADD = mybir.AluOpType.add
    MAX = mybir.AluOpType.max

    LEN = layout["LEN"]
    Tk = layout["Tk"]
    POS = layout["POS"]
    waves = layout["waves"]
    NW = layout["NW"]
    BOFF = layout["BOFF"]
    IOFF = layout["IOFF"]
    LIDX = layout["LIDX"]
    Lpad = layout["Lpad"]

    n_steps_run = int(os.environ.get("GNN_STEPS", str(N_STEPS)))
    n_queues = int(os.environ.get("GNN_QUEUES", "4"))

    nc = bacc.Bacc(get_trn_type() or "TRN2", target_bir_lowering=False,
                   num_swdge_queues=n_queues)

    # ---- I/O -------------------------------------------------------------
    st0_d = nc.dram_tensor("st0", [65, NBP], BF16, kind="ExternalInput")
    bidx_d = nc.dram_tensor("bidx", [128, LIDX // 16], I16, kind="ExternalInput")
    gall_d = nc.dram_tensor("g_all", [128, Lpad], BF16, kind="ExternalInput")
    eall_d = nc.dram_tensor("e_all", [128, Lpad], BF16, kind="ExternalInput")
    molf_d = nc.dram_tensor("molf", [128, NCHUNK], F32, kind="ExternalInput")
    molrow_d = nc.dram_tensor("molrow", [128, 32], I16, kind="ExternalInput")
    iota_d = nc.dram_tensor("iota512", [128, 512], F32, kind="ExternalInput")
    wcat_d = nc.dram_tensor("wcat", [65, N_STEPS, 256], BF16, kind="ExternalInput")
    wcatl_d = nc.dram_tensor("wcat_lo", [65, N_STEPS, 256], BF16, kind="ExternalInput")
    wh_d = nc.dram_tensor("wh", [128, N_STEPS, 128], BF16, kind="ExternalInput")
    whl_d = nc.dram_tensor("wh_lo", [128, N_STEPS, 128], BF16, kind="ExternalInput")
    bh_d = nc.dram_tensor("bh", [128, N_STEPS, 1], F32, kind="ExternalInput")
    wout_d = nc.dram_tensor("wout", [128, N_STEPS, 64], BF16, kind="ExternalInput")
    woutl_d = nc.dram_tensor("wout_lo", [128, N_STEPS, 64], BF16, kind="ExternalInput")
    bout_d = nc.dram_tensor("bout", [64, N_STEPS, 1], F32, kind="ExternalInput")
    wro1_d = nc.dram_tensor("wro1", [65, 256], F32, kind="ExternalInput")
    wroh_d = nc.dram_tensor("wroh", [128, 8, 128], F32, kind="ExternalInput")
    broh_d = nc.dram_tensor("broh", [128, 4, 1], F32, kind="ExternalInput")
    wro2_d = nc.dram_tensor("wro2", [128, 2, 1], F32, kind="ExternalInput")
    bro2_d = nc.dram_tensor("bro2", [1, 1], F32, kind="ExternalInput")
    ident_d = nc.dram_tensor("ident", [128, 128], F32, kind="ExternalInput")
    identb_d = nc.dram_tensor("identb", [64, 64], BF16, kind="ExternalInput")
    identb128_d = nc.dram_tensor("identb128", [128, 128], BF16, kind="ExternalInput")
    out_d = nc.dram_tensor("mol_out", [512], F32, kind="ExternalOutput")
    debug = os.environ.get("GNN_DEBUG", "0") == "1"
    if debug:
        dbgst_d = nc.dram_tensor("dbg_st", [65, NBP], BF16, kind="ExternalOutput")
        dbgasb_d = nc.dram_tensor("dbg_asb", [128, NCHUNK, 128], BF16,
                                  kind="ExternalOutput")
        dbgh1_d = nc.dram_tensor("dbg_h1r", [128, 512], BF16, kind="ExternalOutput")
        dbgbt_d = nc.dram_tensor("dbg_bt", [128, 2048], BF16, kind="ExternalOutput")

    # ---- internal DRAM ---------------------------------------------------
    b_loc = nc.dram_tensor("b_loc", [NBP, 128], BF16)
    b_full = nc.dram_tensor("b_full", [NC * NBP, 128], BF16, addr_space="Shared")
    mol_acc = nc.dram_tensor("mol_acc", [MOLP, 64], F32)
    mol_rs = nc.dram_tensor("mol_rs", [MOL_RS, 64], F32)

    with tile.TileContext(nc) as tc:
        cpool = tc.alloc_tile_pool(name="consts", bufs=1)

        nc.gpsimd.load_library(mlp_lib)

        st_fm = cpool.tile([65, NBP], BF16, tag="st_fm")
        asb = cpool.tile([128, NCHUNK, 128], BF16, tag="asb")
        bidx = cpool.tile([128, LIDX // 16], I16, tag="bidx")
        molf = cpool.tile([128, NCHUNK], F32, tag="molf")
        molrow = cpool.tile([128, 32], I16, tag="molrow")
        iota512 = cpool.tile([128, 512], F32, tag="iota512")
        wcat = cpool.tile([65, N_STEPS, 256], BF16, tag="wcat")
        wcatl = cpool.tile([65, N_STEPS, 256], BF16, tag="wcatl")
        wh = cpool.tile([128, N_STEPS, 128], BF16, tag="wh")
        whl = cpool.tile([128, N_STEPS, 128], BF16, tag="whl")
        bh = cpool.tile([128, N_STEPS, 1], F32, tag="bh")
        wout = cpool.tile([128, N_STEPS, 64], BF16, tag="wout")
        woutl = cpool.tile([128, N_STEPS, 64], BF16, tag="woutl")
        bout = cpool.tile([64, N_STEPS, 1], F32, tag="bout")
        wro1 = cpool.tile([65, 256], F32, tag="wro1")
        wroh = cpool.tile([128, 8, 128], F32, tag="wroh")
        broh = cpool.tile([128, 4, 1], F32, tag="broh")
        wro2 = cpool.tile([128, 2, 1], F32, tag="wro2")
        bro2 = cpool.tile([1, 1], F32, tag="bro2")
        ident = cpool.tile([128, 128], F32, tag="ident")
        identb = cpool.tile([64, 64], BF16, tag="identb")
        identb128 = cpool.tile([128, 128], BF16, tag="identb128")
        zeros = cpool.tile([128, 512], F32, tag="zeros")
        zbf = cpool.tile([1, 512], BF16, tag="zbf")
        nc.vector.memset(zbf[:], 0.0)

        for sb, dr in [(st_fm, st0_d), (bidx, bidx_d), (molf, molf_d),
                       (molrow, molrow_d), (iota512, iota_d), (wcat, wcat_d),
                       (wcatl, wcatl_d), (wh, wh_d), (whl, whl_d),
                       (bh, bh_d), (wout, wout_d), (woutl, woutl_d),
                       (bout, bout_d), (wro1, wro1_d), (wroh, wroh_d),
                       (broh, broh_d), (wro2, wro2_d), (bro2, bro2_d),
                       (ident, ident_d), (identb, identb_d),
                       (identb128, identb128_d)]:
            nc.sync.dma_start(sb[:], dr[:])
        nc.vector.memset(zeros[:], 0.0)

        def zero_dram(dram, nrows):
            full = nrows // 1024
            for r in range(full):
                nc.sync.dma_start(dram[r * 1024:(r + 1) * 1024, :], zeros[:])
            rr = nrows - full * 1024
            if rr:
                nc.sync.dma_start(dram[full * 1024:nrows, :],
                                  zeros[:, 0:(rr * 64) // 128])

        zero_dram(mol_acc, MOLP)

        # molecule psum: persistent across final-step waves
        molpool = tc.alloc_tile_pool(name="molp", bufs=1, space="PSUM")
        mps = molpool.tile([64, 512], F32, tag="mps")
        nc.tensor.matmul(mps[:], zbf[0:1, 0:64], zbf[0:1, 0:512],
                         start=True, stop=False, skip_group_check=True)
        rpool = tc.alloc_tile_pool(name="ro", bufs=2)

        for step in range(n_steps_run):
            final = step == n_steps_run - 1

            # ---- table phase: A chunks (SBUF) + B table -> b_loc ---------
            with tc.tile_pool(name=f"tw{step}", bufs=2) as twp, \
                 tc.tile_pool(name=f"tp{step}", bufs=2, space="PSUM") as tpp:
                for gg in range(NCHUNK // 7):
                    bsb = twp.tile([128, 7, 128], BF16, tag="bsb")
                    for i in range(7):
                        k = gg * 7 + i
                        pab = tpp.tile([128, 256], F32, tag="pab")
                        nc.tensor.matmul(
                            pab[:], st_fm[:, k * 128:(k + 1) * 128],
                            wcat[:, step, :], start=True, stop=False)
                        nc.tensor.matmul(
                            pab[:], st_fm[:, k * 128:(k + 1) * 128],
                            wcatl[:, step, :], start=False, stop=True)
                        nc.vector.tensor_copy(asb[:, k, :], pab[:, 0:128])
                        nc.scalar.activation(bsb[:, i, :], pab[:, 128:256],
                                             COPY)
                    nc.sync.dma_start(
                        b_loc[gg * 896:(gg + 1) * 896, :].rearrange(
                            "(i p) c -> p i c", p=128), bsb[:])
            tc.strict_bb_all_engine_barrier()

            nc.gpsimd.collective_compute(
                "AllGather", mybir.AluOpType.bypass,
                replica_groups=[list(range(NC))],
                ins=[b_loc[:]], outs=[b_full[:]])
            tc.strict_bb_all_engine_barrier()

            # ---- edge phase ---------------------------------------------
            wpool = tc.alloc_tile_pool(name=f"w{step}", bufs=2)
            gpool = tc.alloc_tile_pool(name=f"g{step}", bufs=2)
            ppool = tc.alloc_tile_pool(name=f"p{step}", bufs=2, space="PSUM")
            apool = tc.alloc_tile_pool(name=f"a{step}", bufs=2, space="PSUM")

            for w, wch in enumerate(waves):
                k0 = wch[0]
                wc0, wc1 = POS[k0], POS[wch[-1] + 1]
                CW = wc1 - wc0

                # B gathers (4 groups, one per SWDGE queue)
                btiles = []
                for g in range(NG):
                    nwg = NW[w][g]
                    bt = gpool.tile([128, 1, nwg], BF16, tag=f"bt{g}")
                    nc.gpsimd.dma_gather(
                        bt[:], b_full[g * 2 * NBP:(g + 1) * 2 * NBP, :],
                        bidx[:, IOFF[w][g] // 16:(IOFF[w][g] + nwg) // 16],
                        nwg, nwg, 128, transpose=True,
                        single_packet=False, queue_num=g % n_queues)
                    if debug and step == 0 and w == 0 and g == 0:
                        nc.sync.dma_start(dbgbt_d[:, 0:nwg], bt[:, 0, :])
                    btiles.append(bt)

                # one-hot streams for this wave
                gt = gpool.tile([128, CW], BF16, tag="gt")
                nc.sync.dma_start(gt[:], gall_d[:, wc0:wc1])
                et = gpool.tile([128, CW], BF16, tag="et")
                nc.sync.dma_start(et[:], eall_d[:, wc0:wc1])

                # aggregation psums ([64,512] f32 covers 4 chunks; final
                # step uses row-major [128, 64] per chunk instead)
                if not final:
                    agg = apool.tile([64, len(wch) * 128], F32, tag="agg",
                                     name="agg", bufs=1)
                    for z0 in range(0, len(wch) * 128, 512):
                        z1 = min(z0 + 512, len(wch) * 128)
                        nc.tensor.matmul(agg[:, z0:z1], zbf[0:1, 0:64],
                                         zbf[0:1, 0:z1 - z0], start=True,
                                         stop=False, skip_group_check=True)
                else:
                    agg = apool.tile([128, len(wch) * 64], F32, tag="aggrm",
                                     name="aggrm", bufs=1)
                    nc.tensor.matmul(agg[:], zbf[0:1, 0:128],
                                     zbf[0:1, 0:len(wch) * 64], start=True,
                                     stop=False, skip_group_check=True)

                # per-chunk B-add pieces: (k, g, col0, len, boff)
                badds = {k: [] for k in wch}
                for k in wch:
                    off = 0
                    for g in range(NG):
                        badds[k].append((g, POS[k] - wc0 + off, LEN[k][g],
                                         BOFF[w][g][k]))
                        off += LEN[k][g]

                # process 512-col slices of the wave
                nslice = CW // TS + (1 if CW % TS else 0)
                for s in range(nslice):
                    c0, c1 = s * TS, min((s + 1) * TS, CW)
                    scw = c1 - c0
                    h1 = ppool.tile([128, TS], F32, tag="h1", bufs=2)
                    nc.tensor.matmul(h1[:, 0:scw], zbf[0:1, 0:128],
                                     zbf[0:1, 0:scw], start=True, stop=False,
                                     skip_group_check=True)
                    # A expansion per chunk piece
                    for k in wch:
                        p0, p1 = POS[k] - wc0, POS[k + 1] - wc0
                        i0, i1 = max(p0, c0), min(p1, c1)
                        if i0 >= i1:
                            continue
                        nc.tensor.matmul(
                            h1[:, i0 - c0:i1 - c0], asb[:, k, :],
                            gt[:, i0:i1], start=False, stop=False,
                            skip_group_check=True)
                    # B adds per (chunk, group) piece
                    for k in wch:
                        for g, cstart, ln, boff in badds[k]:
                            i0, i1 = max(cstart, c0), min(cstart + ln, c1)
                            if i0 >= i1:
                                continue
                            bo = boff + (i0 - cstart)
                            nc.tensor.matmul(
                                h1[:, i0 - c0:i1 - c0], identb128[:],
                                btiles[g][:, 0, bo:bo + (i1 - i0)],
                                start=False, stop=True,
                                skip_group_check=True)
                    h1r = wpool.tile([128, TS], BF16, tag="h1r")
                    nc.scalar.activation(h1r[:, 0:scw], h1[:, 0:scw], RELU)
                    if debug and step == 0 and w == 0 and s == 0:
                        nc.sync.dma_start(dbgh1_d[:], h1r[:])
                    p2 = ppool.tile([128, TS], F32, tag="p2", bufs=1)
                    nc.tensor.matmul(p2[:, 0:scw], wh[:, step, :],
                                     h1r[:, 0:scw], start=True, stop=False)
                    nc.tensor.matmul(p2[:, 0:scw], whl[:, step, :],
                                     h1r[:, 0:scw], start=False, stop=True)
                    h2 = wpool.tile([128, TS], BF16, tag="h2")
                    nc.scalar.activation(h2[:, 0:scw], p2[:, 0:scw], RELU,
                                         bias=bh[:, step, :])
                    pm = ppool.tile([64, TS], F32, tag="pm", bufs=1)
                    nc.tensor.matmul(pm[:, 0:scw], wout[:, step, :],
                                     h2[:, 0:scw], start=True, stop=False)
                    nc.tensor.matmul(pm[:, 0:scw], woutl[:, step, :],
                                     h2[:, 0:scw], start=False, stop=True)
                    mt = wpool.tile([64, TS], BF16, tag="mt")
                    nc.vector.tensor_scalar(
                        mt[:, 0:scw], pm[:, 0:scw], bout[:, step, :],
                        0.0, op0=ADD, op1=MAX)
                    # transpose + aggregate per 128-edge subtile
                    for t0 in range(0, scw, 128):
                        gcol = c0 + t0                    # wave-rel col
                        k = int(np.searchsorted(POS, wc0 + gcol, "right")) - 1
                        kl = k - k0
                        pt = ppool.tile([128, 64], BF16, tag="pt", bufs=1)
                        nc.tensor.matmul(pt[:], mt[:, t0:t0 + 128],
                                         identb[:], is_transpose=True,
                                         start=True, stop=True)
                        mem = wpool.tile([128, 64], BF16, tag="mem")
                        nc.vector.tensor_copy(mem[:], pt[:])
                        last = (wc0 + gcol + 128) == POS[k + 1]
                        if not final:
                            nc.tensor.matmul(
                                agg[:, kl * 128:kl * 128 + 128],
                                mem[:], et[:, gcol:gcol + 128],
                                start=False, stop=last,
                                skip_group_check=True)
                        else:
                            nc.tensor.matmul(
                                agg[:, kl * 64:kl * 64 + 64],
                                et[:, gcol:gcol + 128],
                                mem[:], start=False, stop=last,
                                skip_group_check=True)

                # drain aggregation psum
                if not final:
                    nc.vector.tensor_copy(
                        st_fm[0:64, k0 * 128:k0 * 128 + len(wch) * 128],
                        agg[:])
                else:
                    strm = rpool.tile([128, len(wch), 64], BF16, tag="strm")
                    for kl, k in enumerate(wch):
                        nc.vector.tensor_copy(strm[:, kl, :],
                                              agg[:, kl * 64:kl * 64 + 64])
                        moh = wpool.tile([128, 512], BF16, tag="moh")
                        nc.vector.tensor_scalar(
                            moh[:], iota512[:], molf[:, k:k + 1], None,
                            op0=mybir.AluOpType.is_equal)
                        nc.tensor.matmul(mps[:], strm[:, kl, :], moh[:],
                                         start=False, stop=False,
                                         skip_group_check=True)

            if debug and step == 0:
                nc.sync.dma_start(dbgst_d[:], st_fm[:])
                nc.sync.dma_start(dbgasb_d[:], asb[:])

            apool.release()
            ppool.release()
            gpool.release()
            wpool.release()

        # ---- molecule table scatter + ReduceScatter + readout ------------
        mfl = rpool.tile([64, 512], F32, tag="mfl")
        nc.vector.tensor_copy(mfl[:], mps[:])
        molpool.release()
        rpsum = tc.alloc_tile_pool(name="rop", bufs=2, space="PSUM")
        mws = rpool.tile([128, 4, 64], F32, tag="mws")
        for k in range(4):
            ptm = rpsum.tile([128, 64], F32, tag="ptm")
            nc.tensor.matmul(ptm[:], mfl[:, k * 128:(k + 1) * 128],
                             ident[0:64, 0:64], is_transpose=True,
                             start=True, stop=True)
            nc.vector.tensor_copy(mws[:, k, :], ptm[:])
        nc.gpsimd.dma_scatter_add(
            mol_acc[:], mws[:], molrow[:, 0:32],
            512, 512, 64, single_packet=False)
        tc.strict_bb_all_engine_barrier()
        nc.gpsimd.collective_compute(
            "ReduceScatter", mybir.AluOpType.add,
            replica_groups=[list(range(NC))],
            ins=[mol_acc[0:N_MOLS, :]], outs=[mol_rs[:]])
        tc.strict_bb_all_engine_barrier()

        mol_sb = rpool.tile([128, 4, 64], F32, tag="mol_sb")
        nc.sync.dma_start(mol_sb[:, 0:3, :],
                          mol_rs[0:384, :].rearrange(
                              "(i p) c -> p i c", p=128))
        nc.sync.dma_start(mol_sb[0:116, 3, :], mol_rs[384:500, :])
        molt = rpool.tile([65, 512], F32, tag="molt")
        nc.vector.memset(molt[:], 0.0)
        nc.vector.memset(molt[64:65, :], 1.0)
        for k in range(4):
            ptr = rpsum.tile([64, 128], F32, tag="rptr")
            nc.tensor.matmul(ptr[:], mol_sb[:, k, :], ident[:],
                             is_transpose=True, start=True, stop=True)
            nc.vector.tensor_copy(molt[0:64, k * 128:(k + 1) * 128], ptr[:])

        r_cur = rpool.tile([128, 2, 512], F32, tag="r_a")
        for mh in range(2):
            p = rpsum.tile([128, 512], F32, tag="rp")
            nc.tensor.matmul(p[:], wro1[:, mh * 128:(mh + 1) * 128],
                             molt[:], start=True, stop=True)
            nc.scalar.activation(r_cur[:, mh, :], p[:], RELU)
        for layer in range(2):
            r_nxt = rpool.tile([128, 2, 512], F32,
                               tag=("r_b" if layer == 0 else "r_a2"))
            for mh in range(2):
                p = rpsum.tile([128, 512], F32, tag="rp")
                nc.tensor.matmul(p[:], wroh[:, layer * 4 + mh, :], r_cur[:, 0, :],
                                 start=True, stop=False)
                nc.tensor.matmul(p[:], wroh[:, layer * 4 + 2 + mh, :], r_cur[:, 1, :],
                                 start=False, stop=True)
                nc.scalar.activation(r_nxt[:, mh, :], p[:], RELU,
                                     bias=broh[:, layer * 2 + mh, :])
            r_cur = r_nxt
        pout = rpsum.tile([1, 512], F32, tag="rpo")
        nc.tensor.matmul(pout[:], wro2[:, 0, :], r_cur[:, 0, :],
                         start=True, stop=False)
        nc.tensor.matmul(pout[:], wro2[:, 1, :], r_cur[:, 1, :],
                         start=False, stop=True)
        out_sb = rpool.tile([1, 512], F32, tag="out_sb")
        nc.vector.tensor_scalar_add(out_sb[:], pout[:], bro2[:])
        nc.sync.dma_start(out_d[:], out_sb[:])

        rpsum.release()
        rpool.release()
        cpool.release()

    nc.compile()
    return nc


def kernel(**inputs):
    from concourse.bass_utils import run_bass_kernel_spmd

    per_core, layout = _host_prep(inputs)
    w = _weights_prep(inputs)
    nc = _build(layout)

    in_maps = []
    for c in range(NC):
        m = dict(per_core[c])
        m.update(w)
        in_maps.append(m)

    trace = bool(int(os.environ.get("BASS_GNN_TRACE", "0")))
    res = run_bass_kernel_spmd(nc, in_maps, list(range(NC)), trace=trace)
    _LAST["exec_time_ns"] = res.exec_time_ns
    _LAST["results"] = res.results
    _LAST["res"] = res

    out = np.concatenate(
        [np.asarray(res.results[c]["mol_out"][:MOL_RS]) for c in range(NC)])
    return out.astype(np.float32).reshape(N_MOLS, 1)


# revision 35
# speedup vs baseline: 2.1255x; 1.0532x over previous
"""Trainium2 Bass kernel for a GNN message-passing network (v2).

Graph: N=100000 atoms (D=64), E=400000 edges, 3 message-passing steps with a
per-edge MLP (concat(src,dst) -> 128 -> 128 -> 64, relu everywhere), messages
segment-summed at src; then molecule readout (segment-sum by sorted mol_ids ->
MLP 64->256->256->256->1) over M=4000 molecules.

Distribution over 8 NeuronCores (v2 design):
  - atoms partitioned into 8 blocks of NB=12500; core c owns edges whose src
    is in block c. Edges are sorted by (src-chunk of 128 rows, dst 2-bank
    group), padded to a layout that is UNIFORM across cores (so one SPMD
    program works for all 8 cores).
  - per step each core computes A = st@Win[:64]+b and B = st@Win[64:] for its
    own rows (bf16, hi+lo split weights), AllGathers the B table.
  - The per-edge A[src] "gather" is a one-hot matmul: for each 128-row chunk
    k, h1 += A_k^T @ G_k where G_k[row, e] = [src_row(e) == row] is a static
    host-built one-hot streamed from DRAM.  B[dst] is fetched with
    dma_gather (the only dynamic indexing left), split into 4 gathers per
    wave over 2-bank windows (int16-index-safe) on 4 SWDGE queues so
    descriptor generation runs on all 8 Q7 cores, and added into the same
    PSUM tile with an identity matmul.
  - message aggregation at src is another one-hot matmul (E = G^T per
    128-edge subtile) accumulating in PSUM per chunk, writing the new states
    directly in feature-major layout -- no dma_scatter_add, no occurrence
    classes, no inter-step transpose.
  - final step aggregates row-major instead and feeds the molecule one-hot
    matmul (mol_ids sorted => each core block spans < 512 molecules), then
    a unique-index scatter into the global mol table, ReduceScatter, and a
    per-core readout MLP on its 500 molecules; host concatenates.
"""

import os

import ml_dtypes
import numpy as np

N_ATOMS = 100000
N_EDGES = 400000
N_MOLS = 4000
D = 64
DM = 128
N_STEPS = 3
NC = 8
NB = N_ATOMS // NC          # 12500 atoms per core block
NBP = 12544                 # padded to 98*128
NCHUNK = NBP // 128         # 98
NG = 4                      # dst 2-bank gather groups
WV = 8                      # chunks per wave
MOLP = 4608                 # padded mol table
MOL_RS = N_MOLS // NC       # 500 molecules per core after ReduceScatter
TS = 512                    # psum slice width

_LAST = {}


def _wrap16(x, dtype=np.int16):
    """[L] -> [128, L//16] wrapped+replicated gather-index layout."""
    L = x.shape[0]
    assert L % 16 == 0
    w = np.ascontiguousarray(x.reshape(L // 16, 16).T).astype(dtype)
    return np.ascontiguousarray(np.tile(w, (8, 1)))


def _host_prep(inputs):
    """Uniform edge layout + per-core index/one-hot tensors."""
    src = np.asarray(inputs["edge_src"]).astype(np.int64)
    dst = np.asarray(inputs["edge_dst"]).astype(np.int64)
    mol = np.asarray(inputs["mol_ids"]).astype(np.int64)

    core = src // NB
    srcl = src - core * NB
    chunk = srcl // 128
    row = srcl % 128
    bank = dst // NB
    grp = bank // 2
    idx16 = (bank % 2) * NBP + (dst - bank * NB)   # gather idx in 2-bank win
    assert idx16.max() < 2 * NBP <= 32768

    # per (core, chunk, grp) counts -> uniform LEN[k][g] = max over cores
    key = (core * NCHUNK + chunk) * NG + grp
    cnt = np.bincount(key, minlength=NC * NCHUNK * NG).reshape(NC, NCHUNK, NG)
    LEN = cnt.max(axis=0).astype(np.int64)         # [NCHUNK, NG]
    Tk = np.zeros(NCHUNK, dtype=np.int64)
    for k in range(NCHUNK):
        s = int(LEN[k].sum())
        Tk[k] = -(-s // 128) * 128
        LEN[k][NG - 1] += Tk[k] - s                # chunk pad -> last group
    POS = np.zeros(NCHUNK + 1, dtype=np.int64)     # chunk col base (global)
    np.cumsum(Tk, out=POS[1:])
    Lpad = int(POS[-1])

    # waves of WV chunks; per-(wave,grp) gather sizes (128-padded, uniform)
    waves = [list(range(w, min(w + WV, NCHUNK))) for w in range(0, NCHUNK, WV)]
    NW = []                                        # [wave][g] gather num_idxs
    BOFF = []                                      # [wave][g] -> {k: col off}
    for wch in waves:
        nw = []
        boffs = []
        for g in range(NG):
            off = {}
            acc = 0
            for k in wch:
                off[k] = acc
                acc += int(LEN[k][g])
            nw.append(-(-acc // 128) * 128)
            boffs.append(off)
        NW.append(nw)
        BOFF.append(boffs)
    # index-tensor column base per (wave, g), in edges (16-wrapped cols /16)
    IOFF = []
    acc = 0
    for w in range(len(waves)):
        offs = []
        for g in range(NG):
            offs.append(acc)
            acc += NW[w][g]
        IOFF.append(offs)
    LIDX = acc

    # order edges per core into the padded layout
    per_core = []
    for c in range(NC):
        sel = np.nonzero(core == c)[0]
        order = np.lexsort((srcl[sel], grp[sel], chunk[sel]))
        e = sel[order]
        ek, eg, erow, eidx = chunk[e], grp[e], row[e], idx16[e]

        bidx = np.zeros(LIDX, dtype=np.int64)
        G_all = np.zeros((128, Lpad), dtype=np.float32)
        E_all = np.zeros((128, Lpad), dtype=np.float32)
        # run starts per (k,g) in the sorted edge list
        ptr = 0
        for k in range(NCHUNK):
            for g in range(NG):
                n = int(cnt[c, k, g])
                run = slice(ptr, ptr + n)
                ptr += n
                w = k // WV
                # gather slots
                b0 = IOFF[w][g] + BOFF[w][g][k]
                bidx[b0:b0 + n] = eidx[run]
                # global edge positions
                p0 = POS[k] + sum(int(LEN[k][gg]) for gg in range(g))
                p = p0 + np.arange(n)
                r = erow[run]
                G_all[r, p] = 1.0
                E_all[p % 128, (p // 128) * 128 + r] = 1.0
        assert ptr == len(e)
        per_core.append({
            "bidx": _wrap16(bidx),
            "g_all": G_all.astype(ml_dtypes.bfloat16),
            "e_all": E_all.astype(ml_dtypes.bfloat16),
        })

    # molecule one-hot data (as baseline)
    for c in range(NC):
        ms = mol[c * NB:(c + 1) * NB]
        mbase = int(ms[0])
        assert int(ms[-1]) - mbase < 512
        mf = np.full(NBP, 1000.0, dtype=np.float32)
        mf[:NB] = (ms - mbase).astype(np.float32)
        per_core[c]["molf"] = np.ascontiguousarray(
            mf.reshape(NCHUNK, 128).T)                 # [128, 98]
        mrows = (mbase + np.arange(512)).astype(np.int16)
        per_core[c]["molrow"] = _wrap16(mrows)

    # initial states, feature-major + ones row, bf16: [65, NBP]
    st = np.asarray(inputs["atom_states"]).astype(np.float32)
    for c in range(NC):
        s0 = np.zeros((65, NBP), dtype=np.float32)
        s0[:D, :NB] = st[c * NB:(c + 1) * NB].T
        s0[D, :] = 1.0
        per_core[c]["st0"] = s0.astype(ml_dtypes.bfloat16)

    layout = {
        "LEN": LEN.tolist(), "Tk": Tk.tolist(), "POS": POS.tolist(),
        "waves": waves, "NW": NW, "BOFF": BOFF, "IOFF": IOFF,
        "LIDX": LIDX, "Lpad": Lpad,
    }
    return per_core, layout


def _split_bf16(x):
    bf = ml_dtypes.bfloat16
    hi = x.astype(bf)
    lo = (x - hi.astype(np.float32)).astype(bf)
    return hi, lo


def _weights_prep(inputs):
    """Replicated weight tensors (same scheme as baseline)."""
    w = {}
    wcat = np.zeros((N_STEPS, 65, 256), dtype=np.float32)
    for s in range(N_STEPS):
        wi = np.asarray(inputs["ms_W_in"][s], dtype=np.float32)   # [128,128]
        wcat[s, :D, :DM] = wi[:D]
        wcat[s, :D, DM:] = wi[D:]
        wcat[s, D, :DM] = np.asarray(inputs["ms_b_in"][s], dtype=np.float32)
    tp = lambda x: np.ascontiguousarray(np.transpose(x, (1, 0, 2)))
    w["wcat"], w["wcat_lo"] = map(tp, _split_bf16(wcat))
    wh_hi, wh_lo = _split_bf16(np.asarray(inputs["ms_W_h"][:, 0], dtype=np.float32))
    w["wh"], w["wh_lo"] = tp(wh_hi), tp(wh_lo)
    w["bh"] = tp(np.asarray(inputs["ms_b_h"][:, 0], dtype=np.float32)[:, :, None])
    wo_hi, wo_lo = _split_bf16(np.asarray(inputs["ms_W_out"], dtype=np.float32))
    w["wout"], w["wout_lo"] = tp(wo_hi), tp(wo_lo)
    w["bout"] = tp(np.asarray(inputs["ms_b_out"], dtype=np.float32)[:, :, None])

    wro1 = np.zeros((65, 256), dtype=np.float32)
    wro1[:D] = np.asarray(inputs["ro_W_in"], dtype=np.float32)
    wro1[D] = np.asarray(inputs["ro_b_in"], dtype=np.float32)
    w["wro1"] = wro1
    wroh = np.asarray(inputs["ro_W_h"], dtype=np.float32)         # [2,256,256]
    w["wroh"] = np.ascontiguousarray(
        wroh.reshape(2, 2, 128, 2, 128).transpose(2, 0, 1, 3, 4).reshape(128, 8, 128))
    w["broh"] = np.ascontiguousarray(
        np.asarray(inputs["ro_b_h"], dtype=np.float32).reshape(2, 2, 128)
        .transpose(2, 0, 1).reshape(128, 4, 1))
    w["wro2"] = np.ascontiguousarray(
        np.asarray(inputs["ro_W_out"], dtype=np.float32).reshape(2, 128, 1)
        .transpose(1, 0, 2))
    w["bro2"] = np.asarray(
        inputs["ro_b_out"], dtype=np.float32).reshape(1, 1).copy()
    w["ident"] = np.eye(128, dtype=np.float32)
    w["iota512"] = np.tile(np.arange(512, dtype=np.float32), (128, 1))
    w["identb"] = np.eye(64, dtype=np.float32).astype(ml_dtypes.bfloat16)
    w["identb128"] = np.eye(128, dtype=np.float32).astype(ml_dtypes.bfloat16)
    return w


def _build(layout):
    import concourse.bacc as bacc
    import concourse.bass as bass
    import concourse.mybir as mybir
    import concourse.tile as tile
    from concourse._compat import get_trn_type
    from concourse.library_config import mlp as mlp_lib

    F32 = mybir.dt.float32
    BF16 = mybir.dt.bfloat16
    I16 = mybir.dt.int16
    RELU = mybir.ActivationFunctionType.Relu
    COPY = mybir.ActivationFunctionType.Copy
    ADD = mybir.AluOpType.add
    MAX = mybir.AluOpType.max

    LEN = layout["LEN"]
    Tk = layout["Tk"]
    POS = layout["POS"]
    waves = layout["waves"]
    NW = layout["NW"]
    BOFF = layout["BOFF"]
    IOFF = layout["IOFF"]
    LIDX = layout["LIDX"]
    Lpad = layout["Lpad"]

    n_steps_run = int(os.environ.get("GNN_STEPS", str(N_STEPS)))
    n_queues = int(os.environ.get("GNN_QUEUES", "1"))

    nc = bacc.Bacc(get_trn_type() or "TRN2", target_bir_lowering=False,
                   num_swdge_queues=n_queues)

    # ---- I/O -------------------------------------------------------------
    st0_d = nc.dram_tensor("st0", [65, NBP], BF16, kind="ExternalInput")
    bidx_d = nc.dram_tensor("bidx", [128, LIDX // 16], I16, kind="ExternalInput")
    gall_d = nc.dram_tensor("g_all", [128, Lpad], BF16, kind="ExternalInput")
    eall_d = nc.dram_tensor("e_all", [128, Lpad], BF16, kind="ExternalInput")
    molf_d = nc.dram_tensor("molf", [128, NCHUNK], F32, kind="ExternalInput")
    molrow_d = nc.dram_tensor("molrow", [128, 32], I16, kind="ExternalInput")
    iota_d = nc.dram_tensor("iota512", [128, 512], F32, kind="ExternalInput")
    wcat_d = nc.dram_tensor("wcat", [65, N_STEPS, 256], BF16, kind="ExternalInput")
    wcatl_d = nc.dram_tensor("wcat_lo", [65, N_STEPS, 256], BF16, kind="ExternalInput")
    wh_d = nc.dram_tensor("wh", [128, N_STEPS, 128], BF16, kind="ExternalInput")
    whl_d = nc.dram_tensor("wh_lo", [128, N_STEPS, 128], BF16, kind="ExternalInput")
    bh_d = nc.dram_tensor("bh", [128, N_STEPS, 1], F32, kind="ExternalInput")
    wout_d = nc.dram_tensor("wout", [128, N_STEPS, 64], BF16, kind="ExternalInput")
    woutl_d = nc.dram_tensor("wout_lo", [128, N_STEPS, 64], BF16, kind="ExternalInput")
    bout_d = nc.dram_tensor("bout", [64, N_STEPS, 1], F32, kind="ExternalInput")
    wro1_d = nc.dram_tensor("wro1", [65, 256], F32, kind="ExternalInput")
    wroh_d = nc.dram_tensor("wroh", [128, 8, 128], F32, kind="ExternalInput")
    broh_d = nc.dram_tensor("broh", [128, 4, 1], F32, kind="ExternalInput")
    wro2_d = nc.dram_tensor("wro2", [128, 2, 1], F32, kind="ExternalInput")
    bro2_d = nc.dram_tensor("bro2", [1, 1], F32, kind="ExternalInput")
    ident_d = nc.dram_tensor("ident", [128, 128], F32, kind="ExternalInput")
    identb_d = nc.dram_tensor("identb", [64, 64], BF16, kind="ExternalInput")
    identb128_d = nc.dram_tensor("identb128", [128, 128], BF16, kind="ExternalInput")
    out_d = nc.dram_tensor("mol_out", [512], F32, kind="ExternalOutput")
    debug = os.environ.get("GNN_DEBUG", "0") == "1"
    if debug:
        dbgst_d = nc.dram_tensor("dbg_st", [65, NBP], BF16, kind="ExternalOutput")
        dbgasb_d = nc.dram_tensor("dbg_asb", [128, NCHUNK, 128], BF16,
                                  kind="ExternalOutput")
        dbgh1_d = nc.dram_tensor("dbg_h1r", [128, 512], BF16, kind="ExternalOutput")
        dbgbt_d = nc.dram_tensor("dbg_bt", [128, 2048], BF16, kind="ExternalOutput")

    # ---- internal DRAM ---------------------------------------------------
    b_loc = nc.dram_tensor("b_loc", [NBP, 128], BF16)
    b_full = nc.dram_tensor("b_full", [NC * NBP, 128], BF16, addr_space="Shared")
    mol_acc = nc.dram_tensor("mol_acc", [MOLP, 64], F32)
    mol_rs = nc.dram_tensor("mol_rs", [MOL_RS, 64], F32)

    with tile.TileContext(nc) as tc:
        cpool = tc.alloc_tile_pool(name="consts", bufs=1)

        nc.gpsimd.load_library(mlp_lib)

        st_fm = cpool.tile([65, NBP], BF16, tag="st_fm")
        asb = cpool.tile([128, NCHUNK, 128], BF16, tag="asb")
        bidx = cpool.tile([128, LIDX // 16], I16, tag="bidx")
        molf = cpool.tile([128, NCHUNK], F32, tag="molf")
        molrow = cpool.tile([128, 32], I16, tag="molrow")
        iota512 = cpool.tile([128, 512], F32, tag="iota512")
        wcat = cpool.tile([65, N_STEPS, 256], BF16, tag="wcat")
        wcatl = cpool.tile([65, N_STEPS, 256], BF16, tag="wcatl")
        wh = cpool.tile([128, N_STEPS, 128], BF16, tag="wh")
        whl = cpool.tile([128, N_STEPS, 128], BF16, tag="whl")
        bh = cpool.tile([128, N_STEPS, 1], F32, tag="bh")
        wout = cpool.tile([128, N_STEPS, 64], BF16, tag="wout")
        woutl = cpool.tile([128, N_STEPS, 64], BF16, tag="woutl")
        bout = cpool.tile([64, N_STEPS, 1], F32, tag="bout")
        wro1 = cpool.tile([65, 256], F32, tag="wro1")
        wroh = cpool.tile([128, 8, 128], F32, tag="wroh")
        broh = cpool.tile([128, 4, 1], F32, tag="broh")
        wro2 = cpool.tile([128, 2, 1], F32, tag="wro2")
        bro2 = cpool.tile([1, 1], F32, tag="bro2")
        ident = cpool.tile([128, 128], F32, tag="ident")
        identb = cpool.tile([64, 64], BF16, tag="identb")
        identb128 = cpool.tile([128, 128], BF16, tag="identb128")
        zeros = cpool.tile([128, 512], F32, tag="zeros")
        zbf = cpool.tile([1, 512], BF16, tag="zbf")
        nc.vector.memset(zbf[:], 0.0)

        for sb, dr in [(st_fm, st0_d), (bidx, bidx_d), (molf, molf_d),
                       (molrow, molrow_d), (iota512, iota_d), (wcat, wcat_d),
                       (wcatl, wcatl_d), (wh, wh_d), (whl, whl_d),
                       (bh, bh_d), (wout, wout_d), (woutl, woutl_d),
                       (bout, bout_d), (wro1, wro1_d), (wroh, wroh_d),
                       (broh, broh_d), (wro2, wro2_d), (bro2, bro2_d),
                       (ident, ident_d), (identb, identb_d),
                       (identb128, identb128_d)]:
            nc.sync.dma_start(sb[:], dr[:])
        nc.vector.memset(zeros[:], 0.0)

        def zero_dram(dram, nrows):
            full = nrows // 1024
            for r in range(full):
                nc.sync.dma_start(dram[r * 1024:(r + 1) * 1024, :], zeros[:])
            rr = nrows - full * 1024
            if rr:
                nc.sync.dma_start(dram[full * 1024:nrows, :],
                                  zeros[:, 0:(rr * 64) // 128])

        zero_dram(mol_acc, MOLP)

        # molecule psum: persistent across final-step waves
        molpool = tc.alloc_tile_pool(name="molp", bufs=1, space="PSUM")
        mps = molpool.tile([64, 512], F32, tag="mps")
        nc.tensor.matmul(mps[:], zbf[0:1, 0:64], zbf[0:1, 0:512],
                         start=True, stop=False, skip_group_check=True)
        rpool = tc.alloc_tile_pool(name="ro", bufs=2)

        for step in range(n_steps_run):
            final = step == n_steps_run - 1

            # ---- table phase: A chunks (SBUF) + B table -> b_loc ---------
            with tc.tile_pool(name=f"tw{step}", bufs=2) as twp, \
                 tc.tile_pool(name=f"tp{step}", bufs=2, space="PSUM") as tpp:
                for gg in range(NCHUNK // 7):
                    bsb = twp.tile([128, 7, 128], BF16, tag="bsb")
                    for i in range(7):
                        k = gg * 7 + i
                        pab = tpp.tile([128, 256], F32, tag="pab")
                        nc.tensor.matmul(
                            pab[:], st_fm[:, k * 128:(k + 1) * 128],
                            wcat[:, step, :], start=True, stop=False)
                        nc.tensor.matmul(
                            pab[:], st_fm[:, k * 128:(k + 1) * 128],
                            wcatl[:, step, :], start=False, stop=True)
                        nc.vector.tensor_copy(asb[:, k, :], pab[:, 0:128])
                        nc.scalar.activation(bsb[:, i, :], pab[:, 128:256],
                                             COPY)
                    nc.sync.dma_start(
                        b_loc[gg * 896:(gg + 1) * 896, :].rearrange(
                            "(i p) c -> p i c", p=128), bsb[:])
            tc.strict_bb_all_engine_barrier()

            nc.gpsimd.collective_compute(
                "AllGather", mybir.AluOpType.bypass,
                replica_groups=[list(range(NC))],
                ins=[b_loc[:]], outs=[b_full[:]])
            tc.strict_bb_all_engine_barrier()

            # ---- edge phase ---------------------------------------------
            wpool = tc.alloc_tile_pool(name=f"w{step}", bufs=2)
            gpool = tc.alloc_tile_pool(name=f"g{step}", bufs=2)
            ppool = tc.alloc_tile_pool(name=f"p{step}", bufs=2, space="PSUM")
            apool = tc.alloc_tile_pool(name=f"a{step}", bufs=2, space="PSUM")

            for w, wch in enumerate(waves):
                k0 = wch[0]
                wc0, wc1 = POS[k0], POS[wch[-1] + 1]
                CW = wc1 - wc0

                # B gathers (4 groups, one per SWDGE queue)
                btiles = []
                for g in range(NG):
                    nwg = NW[w][g]
                    bt = gpool.tile([128, 1, nwg], BF16, tag=f"bt{g}")
                    nc.gpsimd.dma_gather(
                        bt[:], b_full[g * 2 * NBP:(g + 1) * 2 * NBP, :],
                        bidx[:, IOFF[w][g] // 16:(IOFF[w][g] + nwg) // 16],
                        nwg, nwg, 128, transpose=True,
                        single_packet=False, queue_num=g % n_queues)
                    if debug and step == 0 and w == 0 and g == 0:
                        nc.sync.dma_start(dbgbt_d[:, 0:nwg], bt[:, 0, :])
                    btiles.append(bt)

                # one-hot streams for this wave
                gt = gpool.tile([128, CW], BF16, tag="gt")
                nc.sync.dma_start(gt[:], gall_d[:, wc0:wc1])
                et = gpool.tile([128, CW], BF16, tag="et")
                nc.sync.dma_start(et[:], eall_d[:, wc0:wc1])

                # aggregation psums ([64,512] f32 covers 4 chunks; final
                # step uses row-major [128, 64] per chunk instead)
                if not final:
                    agg = apool.tile([64, len(wch) * 128], F32, tag="agg",
                                     name="agg", bufs=1)
                else:
                    agg = apool.tile([128, len(wch) * 64], F32, tag="aggrm",
                                     name="aggrm", bufs=1)

                # per-chunk B-add pieces: (k, g, col0, len, boff)
                badds = {k: [] for k in wch}
                for k in wch:
                    off = 0
                    for g in range(NG):
                        badds[k].append((g, POS[k] - wc0 + off, LEN[k][g],
                                         BOFF[w][g][k]))
                        off += LEN[k][g]

                # process 512-col slices of the wave
                nslice = CW // TS + (1 if CW % TS else 0)
                for s in range(nslice):
                    c0, c1 = s * TS, min((s + 1) * TS, CW)
                    scw = c1 - c0
                    h1 = ppool.tile([128, TS], F32, tag="h1", bufs=2)
                    # per chunk piece: A expansion (starts the region), then
                    # its B adds -- groups stay contiguous so the bank-wide
                    # has_written clear of the next start=True is harmless
                    for k in wch:
                        p0, p1 = POS[k] - wc0, POS[k + 1] - wc0
                        i0, i1 = max(p0, c0), min(p1, c1)
                        if i0 >= i1:
                            continue
                        nc.tensor.matmul(
                            h1[:, i0 - c0:i1 - c0], asb[:, k, :],
                            gt[:, i0:i1], start=True, stop=False,
                            skip_group_check=True)
                        for g, cstart, ln, boff in badds[k]:
                            j0, j1 = max(cstart, c0), min(cstart + ln, c1)
                            if j0 >= j1:
                                continue
                            bo = boff + (j0 - cstart)
                            nc.tensor.matmul(
                                h1[:, j0 - c0:j1 - c0], identb128[:],
                                btiles[g][:, 0, bo:bo + (j1 - j0)],
                                start=False, stop=True,
                                skip_group_check=True)
                    h1r = wpool.tile([128, TS], BF16, tag="h1r")
                    nc.scalar.activation(h1r[:, 0:scw], h1[:, 0:scw], RELU)
                    if debug and step == 0 and w == 0 and s == 0:
                        nc.sync.dma_start(dbgh1_d[:], h1r[:])
                    p2 = ppool.tile([128, TS], F32, tag="p2", bufs=1)
                    nc.tensor.matmul(p2[:, 0:scw], wh[:, step, :],
                                     h1r[:, 0:scw], start=True, stop=True)
                    h2 = wpool.tile([128, TS], BF16, tag="h2")
                    nc.scalar.activation(h2[:, 0:scw], p2[:, 0:scw], RELU,
                                         bias=bh[:, step, :])
                    pm = ppool.tile([64, TS], F32, tag="pm", bufs=1)
                    nc.tensor.matmul(pm[:, 0:scw], wout[:, step, :],
                                     h2[:, 0:scw], start=True, stop=False)
                    nc.tensor.matmul(pm[:, 0:scw], woutl[:, step, :],
                                     h2[:, 0:scw], start=False, stop=True)
                    mt = wpool.tile([64, TS], BF16, tag="mt")
                    nc.vector.tensor_scalar(
                        mt[:, 0:scw], pm[:, 0:scw], bout[:, step, :],
                        0.0, op0=ADD, op1=MAX)
                    # transpose + aggregate per 128-edge subtile
                    for t0 in range(0, scw, 128):
                        gcol = c0 + t0                    # wave-rel col
                        k = int(np.searchsorted(POS, wc0 + gcol, "right")) - 1
                        kl = k - k0
                        pt = ppool.tile([128, 64], BF16, tag="pt", bufs=1)
                        nc.tensor.matmul(pt[:], mt[:, t0:t0 + 128],
                                         identb[:], is_transpose=True,
                                         start=True, stop=True)
                        mem = wpool.tile([128, 64], BF16, tag="mem")
                        nc.vector.tensor_copy(mem[:], pt[:])
                        first = (wc0 + gcol) == POS[k]
                        last = (wc0 + gcol + 128) == POS[k + 1]
                        if not final:
                            nc.tensor.matmul(
                                agg[:, kl * 128:kl * 128 + 128],
                                mem[:], et[:, gcol:gcol + 128],
                                start=first, stop=last,
                                skip_group_check=True)
                        else:
                            nc.tensor.matmul(
                                agg[:, kl * 64:kl * 64 + 64],
                                et[:, gcol:gcol + 128],
                                mem[:], start=first, stop=last,
                                skip_group_check=True)

                # drain aggregation psum
                if not final:
                    nc.vector.tensor_copy(
                        st_fm[0:64, k0 * 128:k0 * 128 + len(wch) * 128],
                        agg[:])
                else:
                    strm = rpool.tile([128, len(wch), 64], BF16, tag="strm")
                    for kl, k in enumerate(wch):
                        nc.vector.tensor_copy(strm[:, kl, :],
                                              agg[:, kl * 64:kl * 64 + 64])
                        moh = wpool.tile([128, 512], BF16, tag="moh")
                        nc.vector.tensor_scalar(
                            moh[:], iota512[:], molf[:, k:k + 1], None,
                            op0=mybir.AluOpType.is_equal)
                        nc.tensor.matmul(mps[:], strm[:, kl, :], moh[:],
                                         start=False, stop=False,
                                         skip_group_check=True)

            if debug and step == 0:
                nc.sync.dma_start(dbgst_d[:], st_fm[:])
                nc.sync.dma_start(dbgasb_d[:], asb[:])

            apool.release()
            ppool.release()
            gpool.release()
            wpool.release()

        # ---- molecule table scatter + ReduceScatter + readout ------------
        mfl = rpool.tile([64, 512], F32, tag="mfl")
        nc.vector.tensor_copy(mfl[:], mps[:])
        molpool.release()
        rpsum = tc.alloc_tile_pool(name="rop", bufs=2, space="PSUM")
        mws = rpool.tile([128, 4, 64], F32, tag="mws")
        for k in range(4):
            ptm = rpsum.tile([128, 64], F32, tag="ptm")
            nc.tensor.matmul(ptm[:], mfl[:, k * 128:(k + 1) * 128],
                             ident[0:64, 0:64], is_transpose=True,
                             start=True, stop=True)
            nc.vector.tensor_copy(mws[:, k, :], ptm[:])
        nc.gpsimd.dma_scatter_add(
            mol_acc[:], mws[:], molrow[:, 0:32],
            512, 512, 64, single_packet=False)
        tc.strict_bb_all_engine_barrier()
        nc.gpsimd.collective_compute(
            "ReduceScatter", mybir.AluOpType.add,
            replica_groups=[list(range(NC))],
            ins=[mol_acc[0:N_MOLS, :]], outs=[mol_rs[:]])
        tc.strict_bb_all_engine_barrier()

        mol_sb = rpool.tile([128, 4, 64], F32, tag="mol_sb")
        nc.sync.dma_start(mol_sb[:, 0:3, :],
                          mol_rs[0:384, :].rearrange(
                              "(i p) c -> p i c", p=128))
        nc.sync.dma_start(mol_sb[0:116, 3, :], mol_rs[384:500, :])
        molt = rpool.tile([65, 512], F32, tag="molt")
        nc.vector.memset(molt[:], 0.0)
        nc.vector.memset(molt[64:65, :], 1.0)
        for k in range(4):
            ptr = rpsum.tile([64, 128], F32, tag="rptr")
            nc.tensor.matmul(ptr[:], mol_sb[:, k, :], ident[:],
                             is_transpose=True, start=True, stop=True)
            nc.vector.tensor_copy(molt[0:64, k * 128:(k + 1) * 128], ptr[:])

        r_cur = rpool.tile([128, 2, 512], F32, tag="r_a")
        for mh in range(2):
            p = rpsum.tile([128, 512], F32, tag="rp")
            nc.tensor.matmul(p[:], wro1[:, mh * 128:(mh + 1) * 128],
                             molt[:], start=True, stop=True)
            nc.scalar.activation(r_cur[:, mh, :], p[:], RELU)
        for layer in range(2):
            r_nxt = rpool.tile([128, 2, 512], F32,
                               tag=("r_b" if layer == 0 else "r_a2"))
            for mh in range(2):
                p = rpsum.tile([128, 512], F32, tag="rp")
                nc.tensor.matmul(p[:], wroh[:, layer * 4 + mh, :], r_cur[:, 0, :],
                                 start=True, stop=False)
                nc.tensor.matmul(p[:], wroh[:, layer * 4 + 2 + mh, :], r_cur[:, 1, :],
                                 start=False, stop=True)
                nc.scalar.activation(r_nxt[:, mh, :], p[:], RELU,
                                     bias=broh[:, layer * 2 + mh, :])
            r_cur = r_nxt
        pout = rpsum.tile([1, 512], F32, tag="rpo")
        nc.tensor.matmul(pout[:], wro2[:, 0, :], r_cur[:, 0, :],
                         start=True, stop=False)
        nc.tensor.matmul(pout[:], wro2[:, 1, :], r_cur[:, 1, :],
                         start=False, stop=True)
        out_sb = rpool.tile([1, 512], F32, tag="out_sb")
        nc.vector.tensor_scalar_add(out_sb[:], pout[:], bro2[:])
        nc.sync.dma_start(out_d[:], out_sb[:])

        rpsum.release()
        rpool.release()
        cpool.release()

    nc.compile()
    return nc


def kernel(**inputs):
    from concourse.bass_utils import run_bass_kernel_spmd

    per_core, layout = _host_prep(inputs)
    w = _weights_prep(inputs)
    nc = _build(layout)

    in_maps = []
    for c in range(NC):
        m = dict(per_core[c])
        m.update(w)
        in_maps.append(m)

    trace = bool(int(os.environ.get("BASS_GNN_TRACE", "0")))
    res = run_bass_kernel_spmd(nc, in_maps, list(range(NC)), trace=trace)
    _LAST["exec_time_ns"] = res.exec_time_ns
    _LAST["results"] = res.results
    _LAST["res"] = res

    out = np.concatenate(
        [np.asarray(res.results[c]["mol_out"][:MOL_RS]) for c in range(NC)])
    return out.astype(np.float32).reshape(N_MOLS, 1)
